# revision 1
# baseline (speedup 1.0000x reference)
"""GPTQ/ExLlama 4-bit grouped-quantized linear on 8 Trainium2 NeuronCores.

out = x @ dequant(qweight, qzeros, scales) + bias
  x: [4, 2048, 4096] fp16, qweight: [512, 4096] int32 (8 nibbles/int32 along K),
  qzeros: [32, 512] int32 (8 nibbles/int32 along N), scales: [32, 4096] fp16,
  g_idx = arange(K)//128, bias: [4096] fp16.

Sharding: Megatron column-parallel. Each of the 8 cores gets the full x
(replicated) and a 512-wide column slice of qweight/zeros/scales/bias, computes
out[:, n_slice] = x @ W[:, n_slice] + bias[n_slice]; the host concatenates.

Host prep (layout only, no dequant arithmetic): the packed int32 qweight slice
is repacked into uint16 halves laid out so SBUF partition p = 8r+4h+j' holds
half h of packed row r — the device then extracts its nibble with
(q >> 4*j') & 0xF. This turns the weight load into plain contiguous DMAs and
lets the shift/mask run in 16-bit DVE mode.

Per-core kernel:
  - Dequantize W slice [4096, 512] to fp16 in SBUF once (reused by all 64 row
    tiles): (q16 >> shift) & 0xF -> cast fp16 -> subtract z -> multiply s.
  - Stream x through SBUF as transposed tiles [128 k, 32 ko, 256 m] via the
    DMA XBAR transpose, alternating between the two HWDGE rings (sync/scalar);
    matmul-accumulate 32 k-chunks into PSUM [128 m, 512 n]; add bias during the
    PSUM->SBUF copy (DVE); store via SWDGE (gpsimd).
"""

import os
import sys

for _p in ("/opt/trn_rl_repo", "/root/.axon_site/_ro/trn_rl_repo"):
    if os.path.isdir(_p) and _p not in sys.path:
        sys.path.insert(0, _p)

import numpy as np

import concourse.bass as bass
import concourse.mybir as mybir
import concourse.tile as tile
from concourse.bass_utils import run_bass_kernel_spmd
from concourse.tile import add_dep_helper

P = 128                    # partitions
B, S, K, N = 4, 2048, 4096, 4096
M = B * S                  # 8192 rows
GS = 128                   # quant group size (== one k-chunk)
G = K // GS                # 32 groups == k-chunks
NCORES = 8
NC = N // NCORES           # 512 output cols per core
SC = 4                     # groups per dequant super-chunk
NSC = G // SC              # 8 super-chunks
MT = 256                   # x rows per transposed DMA load
NMT = M // MT              # 32 loads
MSUB = MT // P             # 2 psum tiles per load

_built = None


def _split_multiwaits(nc):
    """This container's walrus rejects any instruction carrying more than one
    semaphore wait ("Too many sync wait commands"). Hoist all but one wait of
    each multi-wait instruction into standalone EventSemaphore (wait-only)
    instructions on the same engine, inserted immediately before it — the
    engine queue is FIFO, so semantics are identical."""
    n = 0
    for fn in nc.m.functions:
        for blk in fn.blocks:
            out = []
            for inst in blk.instructions:
                si = getattr(inst, "sync_info", None)
                waits = list(si.on_wait) if si is not None and si.on_wait else []
                if len(waits) > 1:
                    for k, w in enumerate(waits[:-1]):
                        es = mybir.InstEventSemaphore(
                            name=f"{inst.name}.hoistw{k}", ins=[], outs=[],
                            sync_info=mybir.SyncInfo(on_wait=[w], on_update=[]),
                        )
                        es.engine = inst.engine
                        out.append(es)
                        n += 1
                    si.on_wait = [waits[-1]]
                out.append(inst)
            blk.instructions = out
    return n


def _build_bass():
    """Build the (identical-per-core) Bass program once."""
    global _built
    if _built is not None:
        return _built

    nc = bass.Bass()
    x_h = nc.dram_tensor("x", [M, K], mybir.dt.float16, kind="ExternalInput")
    qw16_h = nc.dram_tensor("qw16", [P, G * NC], mybir.dt.uint16, kind="ExternalInput")
    zs_h = nc.dram_tensor("zs", [G, 2, NC], mybir.dt.float16, kind="ExternalInput")
    bias_h = nc.dram_tensor("bias", [NC], mybir.dt.float32, kind="ExternalInput")
    shmk_h = nc.dram_tensor(
        "shmk", [P, 2, SC * NC], mybir.dt.uint16, kind="ExternalInput"
    )
    out_h = nc.dram_tensor("out", [M, NC], mybir.dt.float16, kind="ExternalOutput")

    with tile.TileContext(nc) as tc:
        with (
            tc.tile_pool(name="singles", bufs=1) as singles,
            tc.tile_pool(name="wpool", bufs=NSC) as wpool,
            tc.tile_pool(name="dq", bufs=3) as dq,
            tc.tile_pool(name="xp", bufs=4) as xp,
            tc.tile_pool(name="psum", bufs=8, space="PSUM") as psum,
            tc.tile_pool(name="op", bufs=4) as op,
        ):
            dq_dmas = []
            shmk_t = singles.tile([P, 2, SC * NC], mybir.dt.uint16)
            dq_dmas.append(nc.gpsimd.dma_start(shmk_t[:], shmk_h.ap()))
            # PE warm-up fodder (keeps HAM at full clock while dequant runs)
            wu_w = singles.tile([P, P], mybir.dt.float16)
            nc.vector.memset(wu_w[:], 0.0)
            wu_r = singles.tile([P, NC], mybir.dt.float16)
            nc.vector.memset(wu_r[:], 0.0)

            # ---- dequantize W into SBUF: NSC tiles of [128, SC, NC] fp16 ----
            W_tiles = []
            for sci in range(NSC):
                q16 = dq.tile([P, SC, NC], mybir.dt.uint16, tag="q16")
                d1 = nc.gpsimd.dma_start(
                    q16[:], qw16_h.ap()[:, sci * SC * NC : (sci + 1) * SC * NC]
                )
                zs_t = dq.tile([P, SC, 2, NC], mybir.dt.float16, tag="zs")
                d2 = nc.gpsimd.dma_start(
                    zs_t[:],
                    zs_h.ap()[None, sci * SC : (sci + 1) * SC, :, :].to_broadcast(
                        (P, SC, 2, NC)
                    ),
                )
                if sci == 0:
                    dq_dmas += [d1, d2]

                w_t = wpool.tile([P, SC, NC], mybir.dt.float16, tag="W")
                nc.vector.tensor_tensor(
                    q16[:], q16[:], shmk_t[:, 0, :],
                    mybir.AluOpType.logical_shift_right,
                )
                # bitvec TT ops require src dtype == dst dtype (and bitwise is
                # DVE-only); keep the whole chain on DVE — offloading any link
                # to ACT/gpsimd serializes worse than DVE's own throughput.
                nc.vector.tensor_tensor(
                    q16[:], q16[:], shmk_t[:, 1, :], mybir.AluOpType.bitwise_and
                )
                nc.vector.tensor_copy(out=w_t[:], in_=q16[:])
                nc.vector.tensor_tensor(
                    w_t[:], w_t[:], zs_t[:, :, 0, :], mybir.AluOpType.subtract
                )
                nc.vector.tensor_tensor(
                    w_t[:], w_t[:], zs_t[:, :, 1, :], mybir.AluOpType.mult
                )
                W_tiles.append(w_t)

            # bias is only needed by the first epilogue (~60us in); load late so
            # it doesn't occupy the startup DMA window.
            bias_t = singles.tile([P, NC], mybir.dt.float32)
            nc.gpsimd.dma_start(bias_t[:], bias_h.ap()[None, :].to_broadcast((P, NC)))

            # ---- PE warm-up: dummy matmuls fill the idle window before the
            # first xT tile lands, so real matmuls start at full clock ----
            wu_ps = psum.tile([P, NC], mybir.dt.float32, tag="ps")
            for _ in range(56):
                nc.tensor.matmul(wu_ps[:], wu_w[:], wu_r[:], start=True, stop=True)

            # ---- matmul: stream xT tiles, accumulate over the 32 k-chunks ----
            for mt in range(NMT):
                xt = xp.tile([P, G, MT], mybir.dt.float16)
                nc.sync.dma_start_transpose(
                    xt[:], x_h.ap()[mt * MT : (mt + 1) * MT, :]
                )
                for sub in range(MSUB):
                    ps = psum.tile([P, NC], mybir.dt.float32, tag="ps")
                    for g in range(G):
                        nc.tensor.matmul(
                            ps[:],
                            xt[:, g, sub * P : (sub + 1) * P],
                            W_tiles[g // SC][:, g % SC, :],
                            start=(g == 0),
                            stop=(g == G - 1),
                        )
                    ob = op.tile([P, NC], mybir.dt.float16)
                    nc.vector.tensor_tensor(
                        ob[:], ps[:], bias_t[:], mybir.AluOpType.add
                    )
                    m0 = mt * MT + sub * P
                    # last tile's store on HWDGE: SWDGE completion latency
                    # otherwise sits in the kernel-tail drain.
                    store_eng = nc.sync if mt == NMT - 1 else nc.gpsimd
                    store_eng.dma_start(out_h.ap()[m0 : m0 + P, :], ob[:])

    _split_multiwaits(nc)
    _built = nc
    return nc


def _host_prep(x, qweight, qzeros, scales, bias):
    """Host-side slicing + layout prep (weight repack, zeros unpack, casts).

    qw16 repack: partition p = 8r + 4h + j' (r = packed row within group,
    h = which uint16 half of the int32, j' = nibble within the half) holds
    half h of qweight[16g + r, n] for every group g. All four j' partitions
    share the same source half; they extract different nibbles on-device.
    """
    x2d = np.ascontiguousarray(np.asarray(x).reshape(M, K))
    qweight = np.asarray(qweight)
    qzeros = np.asarray(qzeros)
    scales = np.asarray(scales)
    bias = np.asarray(bias)

    sh8 = (4 * np.arange(8, dtype=np.int64))[None, None, :]
    z = ((qzeros.astype(np.int64)[:, :, None] >> sh8) & 0xF).reshape(G, N) + 1
    zp1 = z.astype(np.float16)

    shmk = np.empty((P, 2, SC * NC), dtype=np.uint16)
    shmk[:, 0, :] = (4 * (np.arange(P, dtype=np.uint16) % 4))[:, None]
    shmk[:, 1, :] = 0xF

    in_maps = []
    for c in range(NCORES):
        n0 = c * NC
        qsl = np.ascontiguousarray(qweight[:, n0 : n0 + NC])       # [K//8, NC]
        u = qsl.view("<u2").reshape(K // 8, NC, 2)                 # [kk, n, h]
        u = u.reshape(G, 16, NC, 2).transpose(1, 3, 0, 2)          # [r, h, g, n]
        qw16 = np.broadcast_to(
            u[:, :, None, :, :], (16, 2, 4, G, NC)
        ).reshape(P, G * NC)
        zs = np.stack(
            [zp1[:, n0 : n0 + NC], scales[:, n0 : n0 + NC].astype(np.float16)],
            axis=1,
        )
        in_maps.append(
            {
                "x": x2d,
                "qw16": np.ascontiguousarray(qw16),
                "zs": np.ascontiguousarray(zs),
                "bias": np.ascontiguousarray(bias[n0 : n0 + NC].astype(np.float32)),
                "shmk": shmk,
            }
        )
    return in_maps


def run(inputs, trace=False, **spmd_kwargs):
    """Run on 8 cores; returns (full_output [4,2048,4096] fp16, BassKernelResults)."""
    nc = _build_bass()
    in_maps = _host_prep(
        inputs["x"], inputs["qweight"], inputs["qzeros"], inputs["scales"],
        inputs["bias"],
    )
    res = run_bass_kernel_spmd(
        nc, in_maps, core_ids=list(range(NCORES)), trace=trace, **spmd_kwargs
    )
    out = np.concatenate([r["out"] for r in res.results], axis=1)
    out = out.reshape(B, S, N).astype(np.float16)
    return out, res


def kernel(x, qweight, qzeros, scales, g_idx, bias):
    out, _ = run(
        {"x": x, "qweight": qweight, "qzeros": qzeros, "scales": scales, "bias": bias}
    )
    return out

